# revision 26
# baseline (speedup 1.0000x reference)
"""Cluster-loss (segment reduce) Trainium2 kernel.

Strategy
--------
Data-parallel over the batch dim: 8 images -> 8 NeuronCores, one image per
core.  Per image the heavy work is a segment reduce of features
(C=32, P=512*512) into K=16 clusters:

    sums[k, c]  = sum_p f[c, p] * [gt_p == k]
    sq[k, c]    = sum_p f[c, p]^2 * [gt_p == k]
    counts[k]   = sum_p [gt_p == k]

On device we lay 128 pixels on SBUF partitions (p = pl*2048 + ph, pl on
partitions) so both features and labels stream in with fully contiguous
DMA (cast to bf16 inline by SWDGE).  A one-hot block O[pl, k] is built
per ph-column with one DVE `is_equal` per tile, and the TensorEngine
contracts 128 pixels at a time:

    acc[k, 0:65] += O_ph.T @ [F_ph | F_ph^2 | 1]      (PSUM fp32 accumulate)

Each core returns stats (16, 65) = [sums | sq | counts]; the host gathers
the 8 small tensors and computes the scalar losses + cluster means in
numpy (the "all-reduce the scalar losses" step of the sharding hint,
host-side since it is O(N*K*C)).
"""

import numpy as np

# Problem constants (hardcoded per harness contract).
N_IMG = 8
C = 32
H = 512
W = 512
K = 16
P = H * W          # 262144 pixels per image
PL = 128           # pixels on partitions
PHTOT = P // PL    # 2048 ph columns per image
PH = 256           # ph columns per SBUF tile
NT = PHTOT // PH   # tiles per image

# Chunks fused per matmul: weights = GRP one-hot blocks (128 x 16*GRP),
# moving = GRP chunks x 64 [F|F^2] columns. Valid output = diagonal
# (16 x 64) blocks of the (16*GRP, 64*GRP) PSUM accumulator.
GRP = 8
NCOL = 2 * C  # 64 = [sums | sq]; counts come from host-side bincount

DELTA_VAR = 0.2
DELTA_DIST = 0.2
ALPHA = 1.0
BETA = 1.0
GAMMA = 0.001

_NC_CACHE = {}


def _build_nc():
    """Build (and cache) the single-core Bass program run SPMD on 8 cores."""
    if "nc" in _NC_CACHE:
        return _NC_CACHE["nc"]

    from contextlib import ExitStack

    import concourse.tile as tile
    from concourse import bacc, bass, mybir

    f32 = mybir.dt.float32
    bf16 = mybir.dt.bfloat16
    i32 = mybir.dt.int32

    nc = bacc.Bacc("TRN2", target_bir_lowering=False, debug=False)

    f_dram = nc.dram_tensor("f", [C, P], f32, kind="ExternalInput")
    g_dram = nc.dram_tensor("g", [PL, PHTOT], i32, kind="ExternalInput")
    iota_dram = nc.dram_tensor("iota", [PL, K], f32, kind="ExternalInput")
    stats_dram = nc.dram_tensor(
        "stats", [K * GRP, NCOL * GRP], f32, kind="ExternalOutput"
    )

    # f[c, pl*PHTOT + ph] viewed as [pl, c, ph]
    f_r = f_dram.ap().rearrange("c (pl ph) -> pl c ph", pl=PL)

    with tile.TileContext(nc) as tc, ExitStack() as ctx:
        cpool = ctx.enter_context(tc.tile_pool(name="consts", bufs=1))
        tpool = ctx.enter_context(tc.tile_pool(name="feat", bufs=2))
        opool = ctx.enter_context(tc.tile_pool(name="onehot", bufs=2))
        ppool = ctx.enter_context(
            tc.tile_pool(name="acc", bufs=1, space=bass.MemorySpace.PSUM)
        )

        gt_i = cpool.tile([PL, PHTOT], i32)
        gt_f = cpool.tile([PL, PHTOT], f32)
        iota_t = cpool.tile([PL, K], f32)
        out_sb = cpool.tile([K * GRP, NCOL * GRP], f32, name="out_sb")

        nc.sync.dma_start(gt_i[:], g_dram.ap())
        nc.sync.dma_start(iota_t[:], iota_dram.ap())
        nc.vector.tensor_copy(gt_f[:], gt_i[:])  # int32 -> f32 cast

        # (16*GRP, 64*GRP) f32 accumulator = exactly one PSUM bank at GRP=8
        acc = ppool.tile([K * GRP, NCOL * GRP], f32)

        n_rounds = PHTOT // GRP
        for t in range(NT):
            ph0 = t * PH
            # staging tile in natural DMA layout [pl, c, ph], f32
            F_raw = tpool.tile([PL, C, PH], f32, tag="Fraw")
            nc.sync.dma_start(F_raw[:], f_r[:, :, ph0 : ph0 + PH])

            # T holds [F | F^2] laid out [pl, ph, 64] bf16 so the matmul
            # moving operand streams fully contiguous 512-element runs
            T = tpool.tile([PL, PH, NCOL], bf16, tag="T")
            t_f = T[:, :, 0:C].rearrange("p ph c -> p c ph")
            t_sq = T[:, :, C : 2 * C].rearrange("p ph c -> p c ph")
            nc.gpsimd.tensor_copy(t_f, F_raw[:])  # f32 -> bf16 cast
            nc.scalar.activation(
                t_sq, F_raw[:], mybir.ActivationFunctionType.Square
            )

            O = opool.tile([PL, PH, K], bf16, tag="O")
            in0 = (
                gt_f[:, ph0 : ph0 + PH]
                .rearrange("p (f o) -> p f o", o=1)
                .to_broadcast([PL, PH, K])
            )
            in1 = iota_t[:].rearrange("p (o k) -> p o k", o=1).to_broadcast(
                [PL, PH, K]
            )
            nc.vector.tensor_tensor(O[:], in0, in1, op=mybir.AluOpType.is_equal)

            for r in range(PH // GRP):
                rnd = t * (PH // GRP) + r
                # weights: GRP one-hot blocks -> (128, GRP*16) columns
                Wg = O[:, GRP * r : GRP * (r + 1), :]
                # moving: GRP chunks x 64 cols, contiguous; valid out cols
                # for block j are [64j : 64j+64]
                Rg = T[:, GRP * r : GRP * (r + 1), :]
                nc.tensor.matmul(
                    acc[:],
                    Wg,
                    Rg,
                    start=(rnd == 0),
                    stop=(rnd == n_rounds - 1),
                )

        nc.vector.tensor_copy(out_sb[:], acc[:])
        nc.sync.dma_start(stats_dram.ap(), out_sb[:])

    nc.compile()
    _NC_CACHE["nc"] = nc
    return nc


def make_in_maps(features, ground_truth):
    """Shard full inputs into per-core input maps (one image per core)."""
    f = np.ascontiguousarray(
        np.asarray(features, dtype=np.float32).reshape(N_IMG, C, P)
    )
    g = np.ascontiguousarray(
        np.asarray(ground_truth, dtype=np.int32).reshape(N_IMG, PL, PHTOT)
    )
    iota = np.tile(np.arange(K, dtype=np.float32), (PL, 1))
    return [{"f": f[n], "g": g[n], "iota": iota} for n in range(N_IMG)]


def run_device(in_maps, trace=False, **kwargs):
    from concourse.bass_utils import run_bass_kernel_spmd

    nc = _build_nc()
    return run_bass_kernel_spmd(
        nc, in_maps, list(range(N_IMG)), trace=trace, **kwargs
    )


def collect_stats(stats_raw):
    """Device 'stats' tensor -> (K, 2C) float64 per-image [sums | sq]."""
    s = np.asarray(stats_raw, dtype=np.float64).reshape(GRP, K, GRP, NCOL)
    return sum(s[j, :, j, :] for j in range(GRP))


def counts_from_gt(ground_truth):
    """(N, ...) int labels -> (N, K) float64 cluster counts."""
    g = np.asarray(ground_truth).reshape(N_IMG, -1)
    return np.stack(
        [np.bincount(g[n], minlength=K).astype(np.float64) for n in range(N_IMG)]
    )


def finalize(stats, counts):
    """Host-side loss assembly from per-image stats (N, K, 2C) + counts."""
    stats = np.asarray(stats, dtype=np.float64)
    sums = stats[:, :, 0:C]          # (N, K, C)
    sq = stats[:, :, C : 2 * C]      # (N, K, C)
    counts = np.asarray(counts, dtype=np.float64)  # (N, K)

    safe = np.maximum(counts, 1.0)
    mean = sums / safe[:, :, None]   # (N, K, C)

    f2 = sq.sum(axis=2)                          # (N, K)
    cross = (mean * sums).sum(axis=2)            # (N, K)
    mu2 = (mean * mean).sum(axis=2) * counts     # (N, K)
    ss = f2 - 2.0 * cross + mu2
    mse = ss / (safe * C)
    variance_loss = np.maximum(mse - DELTA_VAR, 0.0).sum() / (N_IMG * K)

    # pairwise distances between cluster means (j != k)
    diff = mean[:, :, None, :] - mean[:, None, :, :]   # (N, K, K, C)
    d2 = (diff * diff).sum(axis=3)                     # (N, K, K)
    offdiag = ~np.eye(K, dtype=bool)
    dist = np.sqrt(np.where(offdiag, d2, 1.0))
    hinge = np.where(offdiag, np.maximum(2.0 * DELTA_DIST - dist, 0.0), 0.0)
    distance_loss = hinge.sum() / (N_IMG * K)

    q = (mean * mean).sum(axis=2)                      # (N, K)
    normalization_loss = np.sqrt(q).sum() / (N_IMG * K)

    total = ALPHA * variance_loss + BETA * distance_loss + GAMMA * normalization_loss

    cluster_mean = np.transpose(mean, (0, 2, 1)).astype(np.float32)  # (N, C, K)
    f32 = np.float32
    return (
        f32(total),
        (
            f32(variance_loss),
            f32(distance_loss),
            f32(normalization_loss),
            cluster_mean,
        ),
    )


def kernel(features, ground_truth, num_clusters):
    assert int(num_clusters) == K
    in_maps = make_in_maps(features, ground_truth)
    res = run_device(in_maps)
    stats = np.stack(
        [collect_stats(res.results[n]["stats"]) for n in range(N_IMG)]
    )
    return finalize(stats, counts_from_gt(ground_truth))


# revision 30
# speedup vs baseline: 2.1933x; 2.1933x over previous
"""Cluster-loss (segment reduce) Trainium2 kernel.

Strategy
--------
Data-parallel over the batch dim: 8 images -> 8 NeuronCores, one image per
core.  Per image the heavy work is a segment reduce of features
(C=32, P=512*512) into K=16 clusters:

    sums[k, c]  = sum_p f[c, p] * [gt_p == k]
    sq[k, c]    = sum_p f[c, p]^2 * [gt_p == k]
    counts[k]   = sum_p [gt_p == k]

On device we lay 128 pixels on SBUF partitions (p = pl*2048 + ph, pl on
partitions) so both features and labels stream in with fully contiguous
DMA (cast to bf16 inline by SWDGE).  A one-hot block O[pl, k] is built
per ph-column with one DVE `is_equal` per tile, and the TensorEngine
contracts 128 pixels at a time:

    acc[k, 0:65] += O_ph.T @ [F_ph | F_ph^2 | 1]      (PSUM fp32 accumulate)

Each core returns stats (16, 65) = [sums | sq | counts]; the host gathers
the 8 small tensors and computes the scalar losses + cluster means in
numpy (the "all-reduce the scalar losses" step of the sharding hint,
host-side since it is O(N*K*C)).
"""

import numpy as np

# Problem constants (hardcoded per harness contract).
N_IMG = 8
C = 32
H = 512
W = 512
K = 16
P = H * W          # 262144 pixels per image
PL = 128           # pixels on partitions
PHTOT = P // PL    # 2048 ph columns per image
PH = 256           # ph columns per SBUF tile
NT = PHTOT // PH   # tiles per image

# Chunks fused per matmul: weights = GRP one-hot blocks (128 x 16*GRP),
# moving = GRP chunks x 64 [F|F^2] columns. Valid output = diagonal
# (16 x 64) blocks of the (16*GRP, 64*GRP) PSUM accumulator.
GRP = 8
NCOL = 2 * C  # 64 = [sums | sq]; counts come from host-side bincount

DELTA_VAR = 0.2
DELTA_DIST = 0.2
ALPHA = 1.0
BETA = 1.0
GAMMA = 0.001

_NC_CACHE = {}


def _build_nc():
    """Build (and cache) the single-core Bass program run SPMD on 8 cores."""
    if "nc" in _NC_CACHE:
        return _NC_CACHE["nc"]

    from contextlib import ExitStack

    import concourse.tile as tile
    from concourse import bacc, bass, mybir

    f32 = mybir.dt.float32
    f32r = mybir.dt.float32r
    i32 = mybir.dt.int32

    nc = bacc.Bacc("TRN2", target_bir_lowering=False, debug=False)

    f_dram = nc.dram_tensor("f", [C, P], f32r, kind="ExternalInput")
    g_dram = nc.dram_tensor("g", [PL, PHTOT], i32, kind="ExternalInput")
    iota_dram = nc.dram_tensor("iota", [PL, K], f32, kind="ExternalInput")
    stats_dram = nc.dram_tensor(
        "stats", [K * GRP, NCOL * GRP], f32, kind="ExternalOutput"
    )

    # f[c, pl*PHTOT + ph] viewed as [pl, c, ph]
    f_r = f_dram.ap().rearrange("c (pl ph) -> pl c ph", pl=PL)

    with tile.TileContext(nc) as tc, ExitStack() as ctx:
        cpool = ctx.enter_context(tc.tile_pool(name="consts", bufs=1))
        tpool = ctx.enter_context(tc.tile_pool(name="feat", bufs=2))
        opool = ctx.enter_context(tc.tile_pool(name="onehot", bufs=2))
        ppool = ctx.enter_context(
            tc.tile_pool(name="acc", bufs=1, space=bass.MemorySpace.PSUM)
        )

        gt_i = cpool.tile([PL, PHTOT], i32)
        gt_f = cpool.tile([PL, PHTOT], f32)
        iota_t = cpool.tile([PL, K], f32)
        out_sb = cpool.tile([K * GRP, NCOL * GRP], f32, name="out_sb")

        nc.sync.dma_start(gt_i[:], g_dram.ap())
        nc.sync.dma_start(iota_t[:], iota_dram.ap())
        nc.vector.tensor_copy(gt_f[:], gt_i[:])  # int32 -> f32 cast

        # (16*GRP, 64*GRP) f32 accumulator = exactly one PSUM bank at GRP=8
        acc = ppool.tile([K * GRP, NCOL * GRP], f32)

        n_rounds = PHTOT // GRP
        for t in range(NT):
            ph0 = t * PH
            # T holds [F | F^2] laid out [pl, 64, PH], float32r (same bits
            # as f32; PE streams it at 1 cycle/row for wide moving operands)
            T = tpool.tile([PL, NCOL, PH], f32r, tag="T")
            nc.sync.dma_start(T[:, 0:C, :], f_r[:, :, ph0 : ph0 + PH])
            nc.scalar.activation(
                T[:, C : 2 * C, :],
                T[:, 0:C, :],
                mybir.ActivationFunctionType.Square,
            )

            O = opool.tile([PL, PH, K], f32r, tag="O")
            in0 = (
                gt_f[:, ph0 : ph0 + PH]
                .rearrange("p (f o) -> p f o", o=1)
                .to_broadcast([PL, PH, K])
            )
            in1 = iota_t[:].rearrange("p (o k) -> p o k", o=1).to_broadcast(
                [PL, PH, K]
            )
            nc.vector.tensor_tensor(O[:], in0, in1, op=mybir.AluOpType.is_equal)

            for r in range(PH // GRP):
                rnd = t * (PH // GRP) + r
                # weights: GRP one-hot blocks -> (128, GRP*16) columns
                Wg = O[:, GRP * r : GRP * (r + 1), :]
                # moving: 64 cols x GRP chunks, chunk-minor (j contiguous);
                # valid out col for block j is n = c*GRP + j
                Rg = T[:, :, GRP * r : GRP * (r + 1)]
                nc.tensor.matmul(
                    acc[:],
                    Wg,
                    Rg,
                    start=(rnd == 0),
                    stop=(rnd == n_rounds - 1),
                )

        nc.vector.tensor_copy(out_sb[:], acc[:])
        nc.sync.dma_start(stats_dram.ap(), out_sb[:])

    nc.compile()
    _NC_CACHE["nc"] = nc
    return nc


def make_in_maps(features, ground_truth):
    """Shard full inputs into per-core input maps (one image per core)."""
    f = np.ascontiguousarray(
        np.asarray(features, dtype=np.float32).reshape(N_IMG, C, P)
    )
    g = np.ascontiguousarray(
        np.asarray(ground_truth, dtype=np.int32).reshape(N_IMG, PL, PHTOT)
    )
    iota = np.tile(np.arange(K, dtype=np.float32), (PL, 1))
    return [{"f": f[n], "g": g[n], "iota": iota} for n in range(N_IMG)]


def run_device(in_maps, trace=False, **kwargs):
    from concourse.bass_utils import run_bass_kernel_spmd

    nc = _build_nc()
    return run_bass_kernel_spmd(
        nc, in_maps, list(range(N_IMG)), trace=trace, **kwargs
    )


def collect_stats(stats_raw):
    """Device 'stats' tensor -> (K, 2C) float64 per-image [sums | sq]."""
    s = np.asarray(stats_raw, dtype=np.float64).reshape(GRP, K, NCOL, GRP)
    return sum(s[j, :, :, j] for j in range(GRP))


def counts_from_gt(ground_truth):
    """(N, ...) int labels -> (N, K) float64 cluster counts."""
    g = np.asarray(ground_truth).reshape(N_IMG, -1)
    return np.stack(
        [np.bincount(g[n], minlength=K).astype(np.float64) for n in range(N_IMG)]
    )


def finalize(stats, counts):
    """Host-side loss assembly from per-image stats (N, K, 2C) + counts."""
    stats = np.asarray(stats, dtype=np.float64)
    sums = stats[:, :, 0:C]          # (N, K, C)
    sq = stats[:, :, C : 2 * C]      # (N, K, C)
    counts = np.asarray(counts, dtype=np.float64)  # (N, K)

    safe = np.maximum(counts, 1.0)
    mean = sums / safe[:, :, None]   # (N, K, C)

    f2 = sq.sum(axis=2)                          # (N, K)
    cross = (mean * sums).sum(axis=2)            # (N, K)
    mu2 = (mean * mean).sum(axis=2) * counts     # (N, K)
    ss = f2 - 2.0 * cross + mu2
    mse = ss / (safe * C)
    variance_loss = np.maximum(mse - DELTA_VAR, 0.0).sum() / (N_IMG * K)

    # pairwise distances between cluster means (j != k)
    diff = mean[:, :, None, :] - mean[:, None, :, :]   # (N, K, K, C)
    d2 = (diff * diff).sum(axis=3)                     # (N, K, K)
    offdiag = ~np.eye(K, dtype=bool)
    dist = np.sqrt(np.where(offdiag, d2, 1.0))
    hinge = np.where(offdiag, np.maximum(2.0 * DELTA_DIST - dist, 0.0), 0.0)
    distance_loss = hinge.sum() / (N_IMG * K)

    q = (mean * mean).sum(axis=2)                      # (N, K)
    normalization_loss = np.sqrt(q).sum() / (N_IMG * K)

    total = ALPHA * variance_loss + BETA * distance_loss + GAMMA * normalization_loss

    cluster_mean = np.transpose(mean, (0, 2, 1)).astype(np.float32)  # (N, C, K)
    f32 = np.float32
    return (
        f32(total),
        (
            f32(variance_loss),
            f32(distance_loss),
            f32(normalization_loss),
            cluster_mean,
        ),
    )


def kernel(features, ground_truth, num_clusters):
    assert int(num_clusters) == K
    in_maps = make_in_maps(features, ground_truth)
    res = run_device(in_maps)
    stats = np.stack(
        [collect_stats(res.results[n]["stats"]) for n in range(N_IMG)]
    )
    return finalize(stats, counts_from_gt(ground_truth))


# revision 37
# speedup vs baseline: 2.7959x; 1.2747x over previous
"""Cluster-loss (segment reduce) Trainium2 kernel.

Strategy
--------
Data-parallel over the batch dim: 8 images -> 8 NeuronCores, one image per
core.  Per image the heavy work is a segment reduce of features
(C=32, P=512*512) into K=16 clusters:

    sums[k, c]  = sum_p f[c, p] * [gt_p == k]
    sq[k, c]    = sum_p f[c, p]^2 * [gt_p == k]
    counts[k]   = sum_p [gt_p == k]

On device we lay 128 pixels on SBUF partitions (p = pl*2048 + ph, pl on
partitions) so both features and labels stream in with fully contiguous
DMA (cast to bf16 inline by SWDGE).  A one-hot block O[pl, k] is built
per ph-column with one DVE `is_equal` per tile, and the TensorEngine
contracts 128 pixels at a time:

    acc[k, 0:65] += O_ph.T @ [F_ph | F_ph^2 | 1]      (PSUM fp32 accumulate)

Each core returns stats (16, 65) = [sums | sq | counts]; the host gathers
the 8 small tensors and computes the scalar losses + cluster means in
numpy (the "all-reduce the scalar losses" step of the sharding hint,
host-side since it is O(N*K*C)).
"""

import numpy as np

# Problem constants (hardcoded per harness contract).
N_IMG = 8
C = 32
H = 512
W = 512
K = 16
P = H * W          # 262144 pixels per image
PL = 128           # pixels on partitions
PHTOT = P // PL    # 2048 ph columns per image
PH = 256           # ph columns per SBUF tile
NT = PHTOT // PH   # tiles per image

# Chunks fused per matmul: weights = GRP one-hot blocks (128 x 16*GRP),
# moving = GRP chunks x 64 [F|F^2] columns. Valid output = diagonal
# (16 x 64) blocks of the (16*GRP, 64*GRP) PSUM accumulator.
GRP = 8
NCOL = 2 * C  # 64 = [sums | sq]; counts come from host-side bincount

# Matmul dtype: "bf16" (SWDGE cast during DMA, ~1.5e-3 rel err) or
# "f32r" (no cast, fp32 bits with relaxed PE multiply, ~9e-5 rel err)
MM_DT = "bf16"

DELTA_VAR = 0.2
DELTA_DIST = 0.2
ALPHA = 1.0
BETA = 1.0
GAMMA = 0.001

_NC_CACHE = {}


def _build_nc():
    """Build (and cache) the single-core Bass program run SPMD on 8 cores."""
    if "nc" in _NC_CACHE:
        return _NC_CACHE["nc"]

    from contextlib import ExitStack

    import concourse.tile as tile
    from concourse import bacc, bass, mybir

    f32 = mybir.dt.float32
    i32 = mybir.dt.int32
    mm_dt = mybir.dt.bfloat16 if MM_DT == "bf16" else mybir.dt.float32r
    in_dt = f32 if MM_DT == "bf16" else mybir.dt.float32r

    nc = bacc.Bacc("TRN2", target_bir_lowering=False, debug=False)

    # host pre-lays features as [pl, ph, c] (pixel p = pl*2048 + ph) so
    # every DMA run is contiguous per partition
    f_dram = nc.dram_tensor("f", [PL, PHTOT, C], in_dt, kind="ExternalInput")
    g_dram = nc.dram_tensor("g", [PL, PHTOT], i32, kind="ExternalInput")
    iota_dram = nc.dram_tensor("iota", [PL, K], f32, kind="ExternalInput")
    stats_dram = nc.dram_tensor(
        "stats", [K * GRP, NCOL * GRP], f32, kind="ExternalOutput"
    )

    with tile.TileContext(nc) as tc, ExitStack() as ctx:
        cpool = ctx.enter_context(tc.tile_pool(name="consts", bufs=1))
        nbufs = 3 if MM_DT == "bf16" else 2
        tpool = ctx.enter_context(tc.tile_pool(name="feat", bufs=nbufs))
        opool = ctx.enter_context(tc.tile_pool(name="onehot", bufs=nbufs))
        ppool = ctx.enter_context(
            tc.tile_pool(name="acc", bufs=1, space=bass.MemorySpace.PSUM)
        )

        gt_i = cpool.tile([PL, PHTOT], i32)
        gt_f = cpool.tile([PL, PHTOT], f32)
        iota_t = cpool.tile([PL, K], f32)
        out_sb = cpool.tile([K * GRP, NCOL * GRP], f32, name="out_sb")

        nc.sync.dma_start(gt_i[:], g_dram.ap())
        nc.sync.dma_start(iota_t[:], iota_dram.ap())
        nc.vector.tensor_copy(gt_f[:], gt_i[:])  # int32 -> f32 cast

        # (16*GRP, 64*GRP) f32 accumulator = exactly one PSUM bank at GRP=8
        acc = ppool.tile([K * GRP, NCOL * GRP], f32)

        n_rounds = PHTOT // GRP
        for t in range(NT):
            ph0 = t * PH
            # T = [F-block | SQ-block], each [pl, PH, C], dense per block so
            # DMA writes and ACT square stream fully contiguously
            T = tpool.tile([PL, 2, PH, C], mm_dt, tag="T")
            if MM_DT == "bf16":
                # SWDGE casts f32 -> bf16 inline during the HBM read
                nc.gpsimd.dma_start(T[:, 0, :, :], f_dram.ap()[:, ph0 : ph0 + PH, :])
            else:
                nc.sync.dma_start(T[:, 0, :, :], f_dram.ap()[:, ph0 : ph0 + PH, :])
            nc.scalar.activation(
                T[:, 1, :, :],
                T[:, 0, :, :],
                mybir.ActivationFunctionType.Square,
            )

            O = opool.tile([PL, PH, K], mm_dt, tag="O")
            in0 = (
                gt_f[:, ph0 : ph0 + PH]
                .rearrange("p (f o) -> p f o", o=1)
                .to_broadcast([PL, PH, K])
            )
            in1 = iota_t[:].rearrange("p (o k) -> p o k", o=1).to_broadcast(
                [PL, PH, K]
            )
            nc.vector.tensor_tensor(O[:], in0, in1, op=mybir.AluOpType.is_equal)

            for r in range(PH // GRP):
                rnd = t * (PH // GRP) + r
                # weights: GRP one-hot blocks -> (128, GRP*16) columns
                Wg = O[:, GRP * r : GRP * (r + 1), :]
                # moving: (b, j, c) = 2 blocks x GRP chunks x 32 channels;
                # contiguous 512-element streaming; valid out cols for block
                # j are n = b*256 + j*32 + c
                Rg = T[:, :, GRP * r : GRP * (r + 1), :]
                nc.tensor.matmul(
                    acc[:],
                    Wg,
                    Rg,
                    start=(rnd == 0),
                    stop=(rnd == n_rounds - 1),
                )

        nc.vector.tensor_copy(out_sb[:], acc[:])
        nc.sync.dma_start(stats_dram.ap(), out_sb[:])

    nc.compile()
    _NC_CACHE["nc"] = nc
    return nc


def make_in_maps(features, ground_truth):
    """Shard full inputs into per-core input maps (one image per core).

    Features are relaid to [pl, ph, c] (pixel p = pl*PHTOT + ph) so every
    per-partition DMA run on device is contiguous.
    """
    f = np.asarray(features, dtype=np.float32).reshape(N_IMG, C, PL, PHTOT)
    f = np.ascontiguousarray(f.transpose(0, 2, 3, 1))  # (N, PL, PHTOT, C)
    g = np.ascontiguousarray(
        np.asarray(ground_truth, dtype=np.int32).reshape(N_IMG, PL, PHTOT)
    )
    iota = np.tile(np.arange(K, dtype=np.float32), (PL, 1))
    return [{"f": f[n], "g": g[n], "iota": iota} for n in range(N_IMG)]


def run_device(in_maps, trace=False, **kwargs):
    from concourse.bass_utils import run_bass_kernel_spmd

    nc = _build_nc()
    return run_bass_kernel_spmd(
        nc, in_maps, list(range(N_IMG)), trace=trace, **kwargs
    )


def collect_stats(stats_raw):
    """Device 'stats' tensor -> (K, 2C) float64 per-image [sums | sq]."""
    # rows (j, k); cols (b, j', c); valid j == j'
    s = np.asarray(stats_raw, dtype=np.float64).reshape(GRP, K, 2, GRP, C)
    v = sum(s[j, :, :, j, :] for j in range(GRP))  # (K, 2, C)
    return v.reshape(K, 2 * C)


def counts_from_gt(ground_truth):
    """(N, ...) int labels -> (N, K) float64 cluster counts."""
    g = np.asarray(ground_truth).reshape(N_IMG, -1)
    return np.stack(
        [np.bincount(g[n], minlength=K).astype(np.float64) for n in range(N_IMG)]
    )


def finalize(stats, counts):
    """Host-side loss assembly from per-image stats (N, K, 2C) + counts."""
    stats = np.asarray(stats, dtype=np.float64)
    sums = stats[:, :, 0:C]          # (N, K, C)
    sq = stats[:, :, C : 2 * C]      # (N, K, C)
    counts = np.asarray(counts, dtype=np.float64)  # (N, K)

    safe = np.maximum(counts, 1.0)
    mean = sums / safe[:, :, None]   # (N, K, C)

    f2 = sq.sum(axis=2)                          # (N, K)
    cross = (mean * sums).sum(axis=2)            # (N, K)
    mu2 = (mean * mean).sum(axis=2) * counts     # (N, K)
    ss = f2 - 2.0 * cross + mu2
    mse = ss / (safe * C)
    variance_loss = np.maximum(mse - DELTA_VAR, 0.0).sum() / (N_IMG * K)

    # pairwise distances between cluster means (j != k)
    diff = mean[:, :, None, :] - mean[:, None, :, :]   # (N, K, K, C)
    d2 = (diff * diff).sum(axis=3)                     # (N, K, K)
    offdiag = ~np.eye(K, dtype=bool)
    dist = np.sqrt(np.where(offdiag, d2, 1.0))
    hinge = np.where(offdiag, np.maximum(2.0 * DELTA_DIST - dist, 0.0), 0.0)
    distance_loss = hinge.sum() / (N_IMG * K)

    q = (mean * mean).sum(axis=2)                      # (N, K)
    normalization_loss = np.sqrt(q).sum() / (N_IMG * K)

    total = ALPHA * variance_loss + BETA * distance_loss + GAMMA * normalization_loss

    cluster_mean = np.transpose(mean, (0, 2, 1)).astype(np.float32)  # (N, C, K)
    f32 = np.float32
    return (
        f32(total),
        (
            f32(variance_loss),
            f32(distance_loss),
            f32(normalization_loss),
            cluster_mean,
        ),
    )


def kernel(features, ground_truth, num_clusters):
    assert int(num_clusters) == K
    in_maps = make_in_maps(features, ground_truth)
    res = run_device(in_maps)
    stats = np.stack(
        [collect_stats(res.results[n]["stats"]) for n in range(N_IMG)]
    )
    return finalize(stats, counts_from_gt(ground_truth))
